# revision 6
# baseline (speedup 1.0000x reference)
"""MetaMoE Trainium2 kernel: 16 experts sharded 2-per-core across 8 NeuronCores.

Each core computes: shared LayerNorm of x, the (replicated) softmax gate, its two
experts' MLP chains, and the gate-weighted partial sum [B, 2]. The host sums the
8 partials and applies the final mean/var head split.

Layout strategy: activations are kept feature-major ([feature, batch]) so every
GEMM is weight-stationary with the batch streaming as the moving operand; the
final w3 GEMM uses h2 as the stationary operand, which lands the output in
batch-major layout where the gate weights are per-partition scalars.

x arrives from the host already transposed ([IN, B]) and bf16-cast, so the
LayerNorm runs directly in feature-major layout: per 512-column chunk one
strided-view DVE reduce forms sum(x) and sum(x^2) over the 8 k-tiles, GPSIMD
partition_all_reduce finishes the feature-dim reduction, and the normalization
is applied in place as two broadcast-view DVE ops. The tensor engine does no
transpose or stats work. LayerNorm gains are folded into w1 on the host (exact
in fp32); weights are bf16-cast on the host, halving weight DMA traffic. All
small constants (biases, gate w2, w3) ship as two pre-packed [128, n] arrays so
the priority DMA path is two cheap descriptor-light transfers.

The gate-2 and w3 matmuls for a chunk's four 128-row tiles share one PSUM tile
drained by a single DVE op, so tiny-matmul PSUM recycling never stalls the
in-order tensor queue.
"""
import sys
import os

sys.path.insert(0, "/opt/trn_rl_repo")

import numpy as np
import ml_dtypes  # noqa: F401

import concourse.bass as bass  # noqa: F401
import concourse.mybir as mybir
from concourse import bacc
from concourse import bass_isa
from concourse.tile import TileContext
from concourse.bass_utils import run_bass_kernel_spmd

F32 = mybir.dt.float32
BF16 = mybir.dt.bfloat16
AF = mybir.ActivationFunctionType
ALU = mybir.AluOpType
AX = mybir.AxisListType
ROP = bass_isa.ReduceOp

B, IN, HID, G1, E = 4096, 1024, 2048, 256, 16
NCORES = 8
EPL = E // NCORES          # experts per core
NB = B // 128              # 32 batch tiles
NK = IN // 128             # 8 contraction tiles for w1 / gate w1
NM = HID // 128            # 16 m-tiles of h1
KH = HID // 128            # 16 contraction tiles for w2
NG = G1 // 128             # 2 m/k tiles for gate hidden
CH = 512                   # batch chunk (matmul moving free dim)
NCH = B // CH              # 8 chunks
BPC = CH // 128            # 4 b-tiles per chunk
EPS = 1e-5
BF = np.dtype(ml_dtypes.bfloat16)

# packed-constant layouts: cst_f (fp32) / cst_b (bf16), [128, n] p-major
CF_GB1 = 0                     # [2]          gb1[m*128+p]
CF_EB1 = 2                     # [EPL*16]     eb1[e, m*128+p]
CF_EB2 = CF_EB1 + EPL * NM     # [EPL*2]      eb2[e, m2*128+p]
CF_B2 = CF_EB2 + EPL * NG      # [E]          gb2 (replicated over p)
CF_B3 = CF_B2 + E              # [EPL*2]      eb3 (replicated over p)
CF_N = CF_B3 + EPL * 2
CB_GW2 = 0                     # [NG*E]       gw2[k2*128+p, j]
CB_W3 = CB_GW2 + NG * E        # [2*EPL*2]    ew3[e, k3*128+p, t]
CB_N = CB_W3 + 2 * EPL * 2


def build_nc():
    nc = bacc.Bacc(None)

    xT = nc.dram_tensor("xT", [IN, B], BF16, kind="ExternalInput")
    gw1 = nc.dram_tensor("gw1", [IN, G1], BF16, kind="ExternalInput")
    ew1 = nc.dram_tensor("ew1", [EPL, IN, HID], BF16, kind="ExternalInput")
    ew2 = nc.dram_tensor("ew2", [EPL, HID, G1], BF16, kind="ExternalInput")
    cstf = nc.dram_tensor("cstf", [128, CF_N], F32, kind="ExternalInput")
    cstb = nc.dram_tensor("cstb", [128, CB_N], BF16, kind="ExternalInput")
    out = nc.dram_tensor("out", [B, 2], F32, kind="ExternalOutput")

    with TileContext(nc) as tc:
        with (
            tc.tile_pool(name="cpool", bufs=1) as cpool,
            tc.tile_pool(name="stat", bufs=1) as stat,
            tc.tile_pool(name="stage", bufs=2) as stpool,
            tc.tile_pool(name="hpool", bufs=1) as hpool,
            tc.tile_pool(name="psA", bufs=4, space="PSUM") as psA,
            tc.tile_pool(name="psB", bufs=2, space="PSUM") as psB,
            tc.tile_pool(name="psC", bufs=2, space="PSUM") as psC,
        ):
            # ---------------- persistent tiles ----------------
            xnT = cpool.tile([128, NK, B], BF16)             # x^T; normalized in place
            gw1b = cpool.tile([128, NK, G1], BF16)
            w1t = [cpool.tile([128, NK, HID], BF16, tag=f"w1t{e}",
                              name=f"w1t{e}") for e in range(EPL)]
            w2t = [cpool.tile([128, KH, G1], BF16, tag=f"w2t{e}",
                              name=f"w2t{e}") for e in range(EPL)]
            cf = cpool.tile([128, CF_N], F32)
            cb = cpool.tile([128, CB_N], BF16)
            exp_all = cpool.tile([128, NB, E], F32)
            recip_all = cpool.tile([128, NB], F32)
            acc = cpool.tile([128, NB, 2], F32)

            def gb1_ap(m):
                return cf[:, CF_GB1 + m:CF_GB1 + m + 1]

            def eb1_ap(e, m):
                o = CF_EB1 + e * NM + m
                return cf[:, o:o + 1]

            def eb2_ap(e, m2):
                o = CF_EB2 + e * NG + m2
                return cf[:, o:o + 1]

            b2bc = cf[:, CF_B2:CF_B2 + E]

            def b3_ap(e):
                o = CF_B3 + e * 2
                return cf[:, o:o + 2]

            def gw2_ap(k2):
                o = CB_GW2 + k2 * E
                return cb[:, o:o + E]

            def w3_ap(k3, e):
                o = CB_W3 + k3 * EPL * 2 + e * 2
                return cb[:, o:o + 2]

            # ---------------- DMA issue (order = transfer priority) -----------
            # sync queue: constants, gate w1, x^T chunks, w2; scalar queue:
            # expert-0 w1; gpsimd SWDGE: expert-1 w1 (needed latest).
            nc.sync.dma_start(cb[:], cstb[:, :])
            nc.sync.dma_start(cf[:], cstf[:, :])
            nc.sync.dma_start(gw1b[:], gw1.rearrange("(k p) m -> p k m", p=128))
            for k in range(NK):
                nc.scalar.dma_start(w1t[0][:, k], ew1[0, k * 128:(k + 1) * 128, :])
            xTr = xT.rearrange("(k p) b -> p k b", p=128)
            for ch in range(NCH):
                nc.sync.dma_start(xnT[:, :, ch * CH:(ch + 1) * CH],
                                  xTr[:, :, ch * CH:(ch + 1) * CH])
            nc.sync.dma_start(w2t[0][:], ew2[0].rearrange("(k p) m -> p k m", p=128))
            for k in range(NK):
                nc.gpsimd.dma_start(w1t[1][:, k], ew1[1, k * 128:(k + 1) * 128, :])
            nc.sync.dma_start(w2t[1][:], ew2[1].rearrange("(k p) m -> p k m", p=128))

            # ---------------- LayerNorm stats + in-place normalize ------------
            def bcast(ap, n):
                return ap.rearrange("p (u b) -> p u b", u=1).to_broadcast(
                    (128, n, CH))

            def stats_chunk(ch):
                c0 = ch * CH
                xc = xnT[:, :, c0:c0 + CH]
                xv = xc.rearrange("p k b -> p b k")
                sacc = stat.tile([128, CH], F32, tag="sacc")
                qacc = stat.tile([128, CH], F32, tag="qacc")
                sqf = stat.tile([128, NK, CH], BF16, tag="sqf")
                nc.vector.tensor_reduce(sacc[:], xv, axis=AX.X, op=ALU.add)
                nc.vector.tensor_tensor(sqf[:], xc, xc, op=ALU.mult)
                nc.gpsimd.partition_all_reduce(sacc[:], sacc[:], 128, ROP.add)
                nc.vector.tensor_reduce(qacc[:], sqf.rearrange("p k b -> p b k"),
                                        axis=AX.X, op=ALU.add)
                nc.gpsimd.partition_all_reduce(qacc[:], qacc[:], 128, ROP.add)
                # mu = S/IN ; var = Q/IN - mu^2 ; rstd = 1/sqrt(var+eps)
                mu = stat.tile([128, CH], F32, tag="mu")
                nc.vector.tensor_scalar_mul(mu[:], sacc[:], 1.0 / IN)
                var = stat.tile([128, CH], F32, tag="var")
                nc.vector.tensor_tensor(var[:], mu[:], mu[:], op=ALU.mult)
                nc.vector.scalar_tensor_tensor(var[:], qacc[:], 1.0 / IN, var[:],
                                               op0=ALU.mult, op1=ALU.subtract)
                nc.vector.tensor_scalar_add(var[:], var[:], EPS)
                nc.scalar.sqrt(var[:], var[:])
                nc.vector.reciprocal_approx_fast(var[:], var[:])
                mub = stat.tile([128, CH], BF16, tag="mub")
                rsb = stat.tile([128, CH], BF16, tag="rsb")
                nc.vector.tensor_copy(mub[:], mu[:])
                nc.vector.tensor_copy(rsb[:], var[:])
                nc.vector.tensor_tensor(xc, xc, bcast(mub, NK), op=ALU.subtract)
                nc.vector.tensor_tensor(xc, xc, bcast(rsb, NK), op=ALU.mult)

            # ---------------- gate chunk routine ----------------
            def gate_chunk(ch):
                c0 = ch * CH
                bt0 = ch * BPC
                g1s = stpool.tile([128, NG, CH], BF16, tag="g1s")
                for m in range(NG):
                    ps = psB.tile([128, CH], F32, tag="psB")
                    for k in range(NK):
                        nc.tensor.matmul(ps[:], gw1b[:, k, m * 128:(m + 1) * 128],
                                         xnT[:, k, c0:c0 + CH],
                                         start=(k == 0), stop=(k == NK - 1))
                    nc.scalar.activation(g1s[:, m], ps[:], AF.Relu,
                                         bias=gb1_ap(m))
                psg = psC.tile([128, BPC, E], F32, tag="psC")
                for bl in range(BPC):
                    for k2 in range(NG):
                        nc.tensor.matmul(psg[:, bl], g1s[:, k2, bl * 128:(bl + 1) * 128],
                                         gw2_ap(k2),
                                         start=(k2 == 0), stop=(k2 == NG - 1))
                lgs = stpool.tile([128, BPC, E], F32, tag="lgs")
                nc.vector.tensor_tensor(
                    lgs[:], psg[:],
                    b2bc.rearrange("p (u j) -> p u j", u=1).to_broadcast(
                        (128, BPC, E)),
                    op=ALU.add)
                mx4 = stpool.tile([128, BPC], F32, tag="mx4")
                nc.vector.tensor_reduce(mx4[:], lgs[:], axis=AX.X, op=ALU.max,
                                        negate=True)
                sm4 = stpool.tile([128, BPC], F32, tag="sm4")
                for bl in range(BPC):
                    nc.scalar.activation(exp_all[:, bt0 + bl], lgs[:, bl], AF.Exp,
                                         bias=mx4[:, bl:bl + 1],
                                         accum_out=sm4[:, bl:bl + 1])
                nc.vector.reciprocal(recip_all[:, bt0:bt0 + BPC], sm4[:])

            # ---------------- expert chunk routine ----------------
            h1s = hpool.tile([128, NM, CH], BF16)

            def expert_chunk(e, ch):
                c0 = ch * CH
                bt0 = ch * BPC
                for m in range(NM):
                    ps = psA.tile([128, CH], F32, tag="psA")
                    for k in range(NK):
                        nc.tensor.matmul(ps[:], w1t[e][:, k, m * 128:(m + 1) * 128],
                                         xnT[:, k, c0:c0 + CH],
                                         start=(k == 0), stop=(k == NK - 1))
                    nc.scalar.activation(h1s[:, m], ps[:], AF.Relu,
                                         bias=eb1_ap(e, m))
                h2t = stpool.tile([128, NG, CH], BF16, tag="h2s")
                for m2 in range(NG):
                    ps = psB.tile([128, CH], F32, tag="psB")
                    for k2 in range(KH):
                        nc.tensor.matmul(ps[:], w2t[e][:, k2, m2 * 128:(m2 + 1) * 128],
                                         h1s[:, k2],
                                         start=(k2 == 0), stop=(k2 == KH - 1))
                    nc.scalar.activation(h2t[:, m2], ps[:], AF.Relu,
                                         bias=eb2_ap(e, m2))
                ps4 = psC.tile([128, BPC, E], F32, tag="psC")
                for bl in range(BPC):
                    for k3 in range(2):
                        nc.tensor.matmul(ps4[:, bl, 0:2],
                                         h2t[:, k3, bl * 128:(bl + 1) * 128],
                                         w3_ap(k3, e),
                                         start=(k3 == 0), stop=(k3 == 1))
                eo4 = stpool.tile([128, BPC, 2], F32, tag="eo4")
                nc.vector.tensor_tensor(
                    eo4[:], ps4[:, :, 0:2],
                    b3_ap(e).rearrange("p (u t) -> p u t", u=1).to_broadcast(
                        (128, BPC, 2)),
                    op=ALU.add)
                if e == 0:
                    for bl in range(BPC):
                        nc.vector.tensor_scalar_mul(acc[:, bt0 + bl], eo4[:, bl],
                                                    exp_all[:, bt0 + bl, 0:1])
                else:
                    of4 = stpool.tile([128, BPC, 2], F32, tag="of4")
                    for bl in range(BPC):
                        nc.vector.scalar_tensor_tensor(
                            acc[:, bt0 + bl], eo4[:, bl],
                            exp_all[:, bt0 + bl, 1:2], acc[:, bt0 + bl],
                            op0=ALU.mult, op1=ALU.add)
                        nc.vector.tensor_scalar_mul(
                            of4[:, bl], acc[:, bt0 + bl],
                            recip_all[:, bt0 + bl:bt0 + bl + 1])
                    nc.sync.dma_start(
                        out[c0:c0 + CH, :].rearrange("(bl p) t -> p bl t", p=128),
                        of4[:])

            # -------- schedule: stats and gate run ahead of the experts --------
            stats_chunk(0)
            stats_chunk(1)
            gate_chunk(0)
            stats_chunk(2)
            gate_chunk(1)
            stats_chunk(3)
            gate_chunk(2)
            gates_done = 3
            stats_done = 4
            for ch in range(NCH):
                expert_chunk(0, ch)
                if stats_done < NCH:
                    stats_chunk(stats_done)
                    stats_done += 1
                if gates_done < NCH:
                    gate_chunk(gates_done)
                    gates_done += 1
            for ch in range(NCH):
                expert_chunk(1, ch)

    nc.finalize()
    return nc


_NC_CACHE = None


def _get_nc():
    global _NC_CACHE
    if _NC_CACHE is None:
        _NC_CACHE = build_nc()
    return _NC_CACHE


def _pack_consts(gb1f, eb1f, eb2, gb2p, eb3):
    """Pack small per-core constants into [128, n] p-major arrays."""
    cf = np.zeros((128, CF_N), np.float32)
    cf[:, CF_GB1:CF_GB1 + NG] = gb1f.reshape(NG, 128).T
    cf[:, CF_EB1:CF_EB1 + EPL * NM] = (
        eb1f.reshape(EPL, NM, 128).transpose(2, 0, 1).reshape(128, EPL * NM))
    cf[:, CF_EB2:CF_EB2 + EPL * NG] = (
        eb2.reshape(EPL, NG, 128).transpose(2, 0, 1).reshape(128, EPL * NG))
    cf[:, CF_B2:CF_B2 + E] = np.broadcast_to(gb2p, (128, E))
    cf[:, CF_B3:CF_B3 + EPL * 2] = np.broadcast_to(eb3.reshape(-1), (128, EPL * 2))
    return cf


def _pack_consts_b(gw2p, ew3):
    cbm = np.zeros((128, CB_N), BF)
    cbm[:, CB_GW2:CB_GW2 + NG * E] = (
        gw2p.reshape(NG, 128, E).transpose(1, 0, 2).reshape(128, NG * E))
    cbm[:, CB_W3:CB_W3 + 2 * EPL * 2] = (
        ew3.reshape(EPL, 2, 128, 2).transpose(2, 1, 0, 3).reshape(128, 2 * EPL * 2))
    return cbm


def _shard_inputs(inputs):
    """Build the 8 per-core input maps (host-side layout/fold work only:
    slicing, permutation, transpose, dtype cast, and folding the LayerNorm
    affine into w1/b1 — exact in fp32)."""
    f = lambda a: np.asarray(a, dtype=np.float32)
    x = f(inputs["x"])
    g_ln_g, g_ln_b = f(inputs["g_ln_g"]), f(inputs["g_ln_b"])
    g_w1, g_b1 = f(inputs["g_w1"]), f(inputs["g_b1"])
    g_w2, g_b2 = f(inputs["g_w2"]), f(inputs["g_b2"])
    e_ln_g, e_ln_b = f(inputs["e_ln_g"]), f(inputs["e_ln_b"])
    e_w1, e_b1 = f(inputs["e_w1"]), f(inputs["e_b1"])
    e_w2, e_b2 = f(inputs["e_w2"]), f(inputs["e_b2"])
    e_w3, e_b3 = f(inputs["e_w3"]), f(inputs["e_b3"])

    # Fold the per-feature LayerNorm affine through w1: wf = diag(g) @ w1,
    # bf = b1 + beta @ w1. Exact (fp32), removes the on-device fold.
    gw1f = np.ascontiguousarray((g_ln_g[:, None] * g_w1), dtype=BF)
    gb1f = g_b1 + g_ln_b @ g_w1
    ew1f = np.ascontiguousarray(e_ln_g[:, :, None] * e_w1, dtype=BF)
    eb1f = e_b1 + np.einsum("ei,eih->eh", e_ln_b, e_w1)

    xTb = np.ascontiguousarray(x.T, dtype=BF)
    ew2b = np.ascontiguousarray(e_w2, dtype=BF)

    in_maps = []
    for c in range(NCORES):
        lo = c * EPL
        experts = list(range(lo, lo + EPL))
        # permute gate columns so this core's experts are columns 0..EPL-1
        perm = experts + [j for j in range(E) if j not in experts]
        in_maps.append({
            "xT": xTb,
            "gw1": gw1f,
            "ew1": np.ascontiguousarray(ew1f[experts]),
            "ew2": np.ascontiguousarray(ew2b[experts]),
            "cstf": _pack_consts(gb1f, eb1f[experts], e_b2[experts],
                                 g_b2[perm], e_b3[experts]),
            "cstb": _pack_consts_b(
                np.ascontiguousarray(g_w2[:, perm], dtype=BF),
                np.asarray(e_w3[experts], dtype=BF)),
        })
    return in_maps


def _run(inputs, trace=False):
    nc = _get_nc()
    in_maps = _shard_inputs(inputs)
    res = run_bass_kernel_spmd(nc, in_maps, core_ids=list(range(NCORES)),
                               trace=trace)
    return res


def kernel(**inputs):
    res = _run(inputs, trace=bool(os.environ.get("MOE_TRACE")))
    total = np.zeros((B, 2), dtype=np.float64)
    for c in range(NCORES):
        total += res.results[c]["out"].astype(np.float64)
    pred_mean = total[:, 0:1].astype(np.float32)
    pv = np.logaddexp(0.0, total[:, 1:2]) + 1e-6
    pred_var = pv.astype(np.float32)
    kernel.last_exec_time_ns = getattr(res, "exec_time_ns", None)
    return pred_mean, pred_var


kernel.last_exec_time_ns = None


# revision 10
# speedup vs baseline: 1.0633x; 1.0633x over previous
"""MetaMoE Trainium2 kernel: 16 experts sharded 2-per-core across 8 NeuronCores.

Each core computes: shared LayerNorm of x, the (replicated) softmax gate, its two
experts' MLP chains, and the gate-weighted partial sum [B, 2]. The host sums the
8 partials and applies the final mean/var head split.

Layout strategy: activations are kept feature-major ([feature, batch]) so every
GEMM is weight-stationary with the batch streaming as the moving operand; the
final w3 GEMM uses h2 as the stationary operand, which lands the output in
batch-major layout where the gate weights are per-partition scalars.

Everything DMA'd is pre-packed on the host into partition-major [128, n]
layouts so each transfer is 128 contiguous descriptors (descriptor generation
on the queues was the previous lead-in bottleneck). x ships transposed,
chunk-major, bf16; LayerNorm stats run in feature-major layout per 512-column
chunk (contiguous-pair tree reduction over the 8 k-tiles on DVE, GPSIMD
partition_all_reduce across partitions, in-place broadcast-view normalize).
The tensor engine does no transpose/stats work. LayerNorm gains fold into w1
on the host (exact, fp32); gate/w3 biases fold into rank-1 matmuls
(ones-column (x) bias-row) so their PSUM tiles are drained by the scalar
engine's Exp / the accumulation ops directly, and the softmax skips the
max-subtraction (logits are O(0.3); exp is exact-safe in fp32).
"""
import sys
import os

sys.path.insert(0, "/opt/trn_rl_repo")

import numpy as np
import ml_dtypes  # noqa: F401

import concourse.bass as bass  # noqa: F401
import concourse.mybir as mybir
from concourse import bacc
from concourse import bass_isa
from concourse.tile import TileContext
from concourse.bass_utils import run_bass_kernel_spmd

F32 = mybir.dt.float32
BF16 = mybir.dt.bfloat16
AF = mybir.ActivationFunctionType
ALU = mybir.AluOpType
AX = mybir.AxisListType
ROP = bass_isa.ReduceOp

B, IN, HID, G1, E = 4096, 1024, 2048, 256, 16
NCORES = 8
EPL = E // NCORES          # experts per core
NB = B // 128              # 32 batch tiles
NK = IN // 128             # 8 contraction tiles for w1 / gate w1
NM = HID // 128            # 16 m-tiles of h1
KH = HID // 128            # 16 contraction tiles for w2
NG = G1 // 128             # 2 m/k tiles for gate hidden
CH = 512                   # batch chunk (matmul moving free dim)
NCH = B // CH              # 8 chunks
BPC = CH // 128            # 4 b-tiles per chunk
EPS = 1e-5
BF = np.dtype(ml_dtypes.bfloat16)

# packed-constant layouts: cst_f (fp32) / cst_b (bf16), [128, n] p-major
CF_GB1 = 0                     # [NG]         gb1[m*128+p]
CF_EB1 = CF_GB1 + NG           # [EPL*NM]     eb1[e, m*128+p]
CF_EB2 = CF_EB1 + EPL * NM     # [EPL*NG]     eb2[e, m2*128+p]
CF_N = CF_EB2 + EPL * NG
CB_GW2 = 0                     # [NG*E]       gw2[k2*128+p, j]
CB_W3 = CB_GW2 + NG * E        # [2*EPL*2]    ew3[e, k3*128+p, t]
CB_ONES = CB_W3 + 2 * EPL * 2  # [128]        1.0 (rank-1 bias matmul lhsT)
CB_B2X4 = CB_ONES + 128        # [BPC*E]      gb2 tiled 4x
CB_B3X4 = CB_B2X4 + BPC * E    # [EPL*BPC*2]  eb3[e] tiled 4x
CB_N = CB_B3X4 + EPL * BPC * 2


def build_nc():
    nc = bacc.Bacc(None)

    xT = nc.dram_tensor("xT", [128, NCH * NK * CH], BF16, kind="ExternalInput")
    gw1 = nc.dram_tensor("gw1", [128, NK * G1], BF16, kind="ExternalInput")
    ew1 = nc.dram_tensor("ew1", [EPL, IN, HID], BF16, kind="ExternalInput")
    ew2 = nc.dram_tensor("ew2", [EPL, 128, KH * G1], BF16, kind="ExternalInput")
    cstf = nc.dram_tensor("cstf", [128, CF_N], F32, kind="ExternalInput")
    cstb = nc.dram_tensor("cstb", [128, CB_N], BF16, kind="ExternalInput")
    out = nc.dram_tensor("out", [128, NB * 2], F32, kind="ExternalOutput")

    with TileContext(nc) as tc:
        with (
            tc.tile_pool(name="cpool", bufs=1) as cpool,
            tc.tile_pool(name="stat", bufs=1) as stat,
            tc.tile_pool(name="stage", bufs=2) as stpool,
            tc.tile_pool(name="hpool", bufs=1) as hpool,
            tc.tile_pool(name="psA", bufs=4, space="PSUM") as psA,
            tc.tile_pool(name="psB", bufs=2, space="PSUM") as psB,
            tc.tile_pool(name="psC", bufs=2, space="PSUM") as psC,
        ):
            # ---------------- persistent tiles ----------------
            xnT = cpool.tile([128, NCH, NK, CH], BF16)       # x^T; normalized in place
            gw1b = cpool.tile([128, NK, G1], BF16)
            w1t = [cpool.tile([128, NK, HID], BF16, tag=f"w1t{e}",
                              name=f"w1t{e}") for e in range(EPL)]
            w2t = [cpool.tile([128, KH, G1], BF16, tag=f"w2t{e}",
                              name=f"w2t{e}") for e in range(EPL)]
            cf = cpool.tile([128, CF_N], F32)
            cb = cpool.tile([128, CB_N], BF16)
            exp_all = cpool.tile([128, NB, E], F32)
            recip_all = cpool.tile([128, NB], F32)
            acc = cpool.tile([128, NB, 2], F32)

            def gb1_ap(m):
                return cf[:, CF_GB1 + m:CF_GB1 + m + 1]

            def eb1_ap(e, m):
                o = CF_EB1 + e * NM + m
                return cf[:, o:o + 1]

            def eb2_ap(e, m2):
                o = CF_EB2 + e * NG + m2
                return cf[:, o:o + 1]

            def gw2_ap(k2):
                o = CB_GW2 + k2 * E
                return cb[:, o:o + E]

            def w3_ap(k3, e):
                o = CB_W3 + k3 * EPL * 2 + e * 2
                return cb[:, o:o + 2]

            ones_row = cb[0:1, CB_ONES:CB_ONES + 128]
            b2_row = cb[0:1, CB_B2X4:CB_B2X4 + E]

            def b3_row(e):
                o = CB_B3X4 + e * BPC * 2
                return cb[0:1, o:o + 2]

            # ---------------- DMA issue (order = transfer priority) -----------
            # sync queue: constants, gate w1, x^T chunks, w2, outputs; scalar
            # queue: expert-0 w1; gpsimd SWDGE: expert-1 w1 (needed latest).
            nc.sync.dma_start(cb[:], cstb[:, :])
            nc.sync.dma_start(cf[:], cstf[:, :])
            nc.sync.dma_start(gw1b[:], gw1.rearrange("p (k m) -> p k m", k=NK))
            for k in range(NK):
                nc.scalar.dma_start(w1t[0][:, k], ew1[0, k * 128:(k + 1) * 128, :])
            xTr = xT.rearrange("p (c k b) -> p c k b", c=NCH, k=NK)
            for ch in range(NCH):
                nc.sync.dma_start(xnT[:, ch], xTr[:, ch])
            nc.sync.dma_start(w2t[0][:], ew2[0].rearrange("p (k m) -> p k m", k=KH))
            for k in range(NK):
                nc.gpsimd.dma_start(w1t[1][:, k], ew1[1, k * 128:(k + 1) * 128, :])
            nc.sync.dma_start(w2t[1][:], ew2[1].rearrange("p (k m) -> p k m", k=KH))

            # ---------------- LayerNorm stats + in-place normalize ------------
            def bcast(ap, n):
                return ap.rearrange("p (u b) -> p u b", u=1).to_broadcast(
                    (128, n, CH))

            def tree_sum(dst, src, t1b):
                """dst[128, CH](f32) = sum over k of src[128, NK, CH] (bf16)."""
                nc.vector.tensor_tensor(t1b[:], src[:, 0:4], src[:, 4:8],
                                        op=ALU.add)
                nc.vector.tensor_tensor(t1b[:, 0:2], t1b[:, 0:2], t1b[:, 2:4],
                                        op=ALU.add)
                nc.vector.tensor_tensor(dst[:], t1b[:, 0], t1b[:, 1], op=ALU.add)

            def stats_chunk(ch):
                xc = xnT[:, ch]
                sacc = stat.tile([128, CH], F32, tag="sacc")
                qacc = stat.tile([128, CH], F32, tag="qacc")
                sqf = stat.tile([128, NK, CH], BF16, tag="sqf")
                t1b = stat.tile([128, 4, CH], BF16, tag="t1b")
                tree_sum(sacc, xc, t1b)
                nc.vector.tensor_tensor(sqf[:], xc, xc, op=ALU.mult)
                nc.gpsimd.partition_all_reduce(sacc[:], sacc[:], 128, ROP.add)
                tree_sum(qacc, sqf, t1b)
                nc.gpsimd.partition_all_reduce(qacc[:], qacc[:], 128, ROP.add)
                # mu = S/IN ; var = Q/IN - mu^2 ; rstd = 1/sqrt(var+eps)
                mu = stat.tile([128, CH], F32, tag="mu")
                nc.vector.tensor_scalar_mul(mu[:], sacc[:], 1.0 / IN)
                var = stat.tile([128, CH], F32, tag="var")
                nc.vector.tensor_tensor(var[:], mu[:], mu[:], op=ALU.mult)
                nc.vector.scalar_tensor_tensor(var[:], qacc[:], 1.0 / IN, var[:],
                                               op0=ALU.mult, op1=ALU.subtract)
                nc.vector.tensor_scalar_add(var[:], var[:], EPS)
                nc.scalar.sqrt(var[:], var[:])
                nc.vector.reciprocal_approx_fast(var[:], var[:])
                mub = stat.tile([128, CH], BF16, tag="mub")
                rsb = stat.tile([128, CH], BF16, tag="rsb")
                nc.vector.tensor_copy(mub[:], mu[:])
                nc.vector.tensor_copy(rsb[:], var[:])
                nc.vector.tensor_tensor(xc, xc, bcast(mub, NK), op=ALU.subtract)
                nc.vector.tensor_tensor(xc, xc, bcast(rsb, NK), op=ALU.mult)

            # ---------------- gate chunk routine ----------------
            def gate_chunk(ch):
                bt0 = ch * BPC
                g1s = stpool.tile([128, NG, CH], BF16, tag="g1s")
                for m in range(NG):
                    ps = psB.tile([128, CH], F32, tag="psB")
                    for k in range(NK):
                        nc.tensor.matmul(ps[:], gw1b[:, k, m * 128:(m + 1) * 128],
                                         xnT[:, ch, k], start=(k == 0),
                                         stop=(k == NK - 1))
                    nc.scalar.activation(g1s[:, m], ps[:], AF.Relu,
                                         bias=gb1_ap(m))
                psg = psC.tile([128, BPC, E], F32, tag="psC")
                for bl in range(BPC):
                    for k2 in range(NG):
                        nc.tensor.matmul(psg[:, bl],
                                         g1s[:, k2, bl * 128:(bl + 1) * 128],
                                         gw2_ap(k2), start=(k2 == 0), stop=False)
                    nc.tensor.matmul(psg[:, bl], ones_row, b2_row,
                                     start=False, stop=True)
                sm4 = stpool.tile([128, BPC], F32, tag="sm4")
                for bl in range(BPC):
                    nc.scalar.activation(exp_all[:, bt0 + bl], psg[:, bl], AF.Exp,
                                         accum_out=sm4[:, bl:bl + 1])
                nc.vector.reciprocal(recip_all[:, bt0:bt0 + BPC], sm4[:])

            # ---------------- expert chunk routine ----------------
            h1s = hpool.tile([128, NM, CH], BF16)

            def expert_chunk(e, ch):
                bt0 = ch * BPC
                for m in range(NM):
                    ps = psA.tile([128, CH], F32, tag="psA")
                    for k in range(NK):
                        nc.tensor.matmul(ps[:], w1t[e][:, k, m * 128:(m + 1) * 128],
                                         xnT[:, ch, k], start=(k == 0),
                                         stop=(k == NK - 1))
                    nc.scalar.activation(h1s[:, m], ps[:], AF.Relu,
                                         bias=eb1_ap(e, m))
                h2t = stpool.tile([128, NG, CH], BF16, tag="h2s")
                for m2 in range(NG):
                    ps = psB.tile([128, CH], F32, tag="psB")
                    for k2 in range(KH):
                        nc.tensor.matmul(ps[:], w2t[e][:, k2, m2 * 128:(m2 + 1) * 128],
                                         h1s[:, k2],
                                         start=(k2 == 0), stop=(k2 == KH - 1))
                    nc.scalar.activation(h2t[:, m2], ps[:], AF.Relu,
                                         bias=eb2_ap(e, m2))
                ps4 = psC.tile([128, BPC, E], F32, tag="psC")
                for bl in range(BPC):
                    for k3 in range(2):
                        nc.tensor.matmul(ps4[:, bl, 0:2],
                                         h2t[:, k3, bl * 128:(bl + 1) * 128],
                                         w3_ap(k3, e), start=(k3 == 0), stop=False)
                    nc.tensor.matmul(ps4[:, bl, 0:2], ones_row, b3_row(e),
                                     start=False, stop=True)
                if e == 0:
                    for bl in range(BPC):
                        nc.vector.tensor_scalar_mul(acc[:, bt0 + bl],
                                                    ps4[:, bl, 0:2],
                                                    exp_all[:, bt0 + bl, 0:1])
                else:
                    of4 = stpool.tile([128, BPC, 2], F32, tag="of4")
                    for bl in range(BPC):
                        nc.vector.scalar_tensor_tensor(
                            acc[:, bt0 + bl], ps4[:, bl, 0:2],
                            exp_all[:, bt0 + bl, 1:2], acc[:, bt0 + bl],
                            op0=ALU.mult, op1=ALU.add)
                        nc.vector.tensor_scalar_mul(
                            of4[:, bl], acc[:, bt0 + bl],
                            recip_all[:, bt0 + bl:bt0 + bl + 1])
                    nc.sync.dma_start(out[:, bt0 * 2:(bt0 + BPC) * 2], of4[:])

            # -------- schedule: stats and gate run ahead of the experts --------
            stats_chunk(0)
            stats_chunk(1)
            gate_chunk(0)
            stats_chunk(2)
            gate_chunk(1)
            gates_done = 2
            stats_done = 3
            for ch in range(NCH):
                expert_chunk(0, ch)
                if stats_done < NCH:
                    stats_chunk(stats_done)
                    stats_done += 1
                if gates_done < NCH:
                    gate_chunk(gates_done)
                    gates_done += 1
            for ch in range(NCH):
                expert_chunk(1, ch)

    nc.finalize()
    return nc


_NC_CACHE = None


def _get_nc():
    global _NC_CACHE
    if _NC_CACHE is None:
        _NC_CACHE = build_nc()
    return _NC_CACHE


def _pack_consts(gb1f, eb1f, eb2):
    """Pack small per-core fp32 constants into a [128, n] p-major array."""
    cfm = np.zeros((128, CF_N), np.float32)
    cfm[:, CF_GB1:CF_GB1 + NG] = gb1f.reshape(NG, 128).T
    cfm[:, CF_EB1:CF_EB1 + EPL * NM] = (
        eb1f.reshape(EPL, NM, 128).transpose(2, 0, 1).reshape(128, EPL * NM))
    cfm[:, CF_EB2:CF_EB2 + EPL * NG] = (
        eb2.reshape(EPL, NG, 128).transpose(2, 0, 1).reshape(128, EPL * NG))
    return cfm


def _pack_consts_b(gw2p, ew3, gb2p, eb3):
    cbm = np.zeros((128, CB_N), BF)
    cbm[:, CB_GW2:CB_GW2 + NG * E] = (
        gw2p.reshape(NG, 128, E).transpose(1, 0, 2).reshape(128, NG * E))
    cbm[:, CB_W3:CB_W3 + 2 * EPL * 2] = (
        ew3.reshape(EPL, 2, 128, 2).transpose(2, 1, 0, 3).reshape(128, 2 * EPL * 2))
    cbm[:, CB_ONES:CB_ONES + 128] = 1.0
    cbm[0, CB_B2X4:CB_B2X4 + BPC * E] = np.tile(gb2p, BPC).astype(BF)
    cbm[0, CB_B3X4:CB_B3X4 + EPL * BPC * 2] = np.tile(
        eb3.reshape(EPL, 1, 2), (1, BPC, 1)).reshape(-1).astype(BF)
    return cbm


def _shard_inputs(inputs):
    """Build the 8 per-core input maps (host-side layout/fold work only:
    slicing, permutation, transpose, dtype cast, and folding the LayerNorm
    affine into w1/b1 — exact in fp32)."""
    f = lambda a: np.asarray(a, dtype=np.float32)
    x = f(inputs["x"])
    g_ln_g, g_ln_b = f(inputs["g_ln_g"]), f(inputs["g_ln_b"])
    g_w1, g_b1 = f(inputs["g_w1"]), f(inputs["g_b1"])
    g_w2, g_b2 = f(inputs["g_w2"]), f(inputs["g_b2"])
    e_ln_g, e_ln_b = f(inputs["e_ln_g"]), f(inputs["e_ln_b"])
    e_w1, e_b1 = f(inputs["e_w1"]), f(inputs["e_b1"])
    e_w2, e_b2 = f(inputs["e_w2"]), f(inputs["e_b2"])
    e_w3, e_b3 = f(inputs["e_w3"]), f(inputs["e_b3"])

    # Fold the per-feature LayerNorm affine through w1: wf = diag(g) @ w1,
    # bf = b1 + beta @ w1. Exact (fp32), removes the on-device fold.
    gw1f = g_ln_g[:, None] * g_w1
    gb1f = g_b1 + g_ln_b @ g_w1
    ew1f = e_ln_g[:, :, None] * e_w1
    eb1f = e_b1 + np.einsum("ei,eih->eh", e_ln_b, e_w1)

    # partition-major packs: every DMA is 128 contiguous runs
    xTp = np.ascontiguousarray(
        x.reshape(NCH, CH, NK, 128).transpose(3, 0, 2, 1).reshape(
            128, NCH * NK * CH), dtype=BF)
    gw1p = np.ascontiguousarray(
        gw1f.reshape(NK, 128, G1).transpose(1, 0, 2).reshape(128, NK * G1),
        dtype=BF)
    ew1b = np.ascontiguousarray(ew1f, dtype=BF)
    ew2p = np.ascontiguousarray(
        e_w2.reshape(E, KH, 128, G1).transpose(0, 2, 1, 3).reshape(
            E, 128, KH * G1), dtype=BF)

    in_maps = []
    for c in range(NCORES):
        lo = c * EPL
        experts = list(range(lo, lo + EPL))
        # permute gate columns so this core's experts are columns 0..EPL-1
        perm = experts + [j for j in range(E) if j not in experts]
        in_maps.append({
            "xT": xTp,
            "gw1": gw1p,
            "ew1": np.ascontiguousarray(ew1b[experts]),
            "ew2": np.ascontiguousarray(ew2p[experts]),
            "cstf": _pack_consts(gb1f, eb1f[experts], e_b2[experts]),
            "cstb": _pack_consts_b(
                np.asarray(g_w2[:, perm], dtype=BF),
                np.asarray(e_w3[experts], dtype=BF),
                np.asarray(g_b2[perm], dtype=BF),
                np.asarray(e_b3[experts], dtype=BF)),
        })
    return in_maps


def _run(inputs, trace=False):
    nc = _get_nc()
    in_maps = _shard_inputs(inputs)
    res = run_bass_kernel_spmd(nc, in_maps, core_ids=list(range(NCORES)),
                               trace=trace)
    return res


def kernel(**inputs):
    res = _run(inputs, trace=bool(os.environ.get("MOE_TRACE")))
    total = np.zeros((B, 2), dtype=np.float64)
    for c in range(NCORES):
        # device output is [128, NB, 2] p-major; restore batch-major [B, 2]
        o = res.results[c]["out"].reshape(128, NB, 2).transpose(1, 0, 2)
        total += o.reshape(B, 2).astype(np.float64)
    pred_mean = total[:, 0:1].astype(np.float32)
    pv = np.logaddexp(0.0, total[:, 1:2]) + 1e-6
    pred_var = pv.astype(np.float32)
    kernel.last_exec_time_ns = getattr(res, "exec_time_ns", None)
    return pred_mean, pred_var


kernel.last_exec_time_ns = None


# revision 11
# speedup vs baseline: 1.1016x; 1.0361x over previous
"""MetaMoE Trainium2 kernel: 16 experts sharded 2-per-core across 8 NeuronCores.

Each core computes: shared LayerNorm of x, the (replicated) softmax gate, its two
experts' MLP chains, and the gate-weighted partial sum [B, 2]. The host sums the
8 partials and applies the final mean/var head split.

Layout strategy: activations are kept feature-major ([feature, batch]) so every
GEMM is weight-stationary with the batch streaming as the moving operand; the
final w3 GEMM uses h2 as the stationary operand, which lands the output in
batch-major layout where the gate weights are per-partition scalars.

Everything DMA'd is pre-packed on the host into partition-major [128, n]
layouts so each transfer is 128 contiguous descriptors (descriptor generation
on the queues was the previous lead-in bottleneck). x ships transposed,
chunk-major, bf16; LayerNorm stats run in feature-major layout per 512-column
chunk (contiguous-pair tree reduction over the 8 k-tiles on DVE, GPSIMD
partition_all_reduce across partitions, in-place broadcast-view normalize).
The tensor engine does no transpose/stats work. LayerNorm gains fold into w1
on the host (exact, fp32); gate/w3 biases fold into rank-1 matmuls
(ones-column (x) bias-row) so their PSUM tiles are drained by the scalar
engine's Exp / the accumulation ops directly, and the softmax skips the
max-subtraction (logits are O(0.3); exp is exact-safe in fp32).
"""
import sys
import os

sys.path.insert(0, "/opt/trn_rl_repo")

import numpy as np
import ml_dtypes  # noqa: F401

import concourse.bass as bass  # noqa: F401
import concourse.mybir as mybir
from concourse import bacc
from concourse import bass_isa
from concourse.tile import TileContext
from concourse.bass_utils import run_bass_kernel_spmd

F32 = mybir.dt.float32
BF16 = mybir.dt.bfloat16
AF = mybir.ActivationFunctionType
ALU = mybir.AluOpType
AX = mybir.AxisListType
ROP = bass_isa.ReduceOp

B, IN, HID, G1, E = 4096, 1024, 2048, 256, 16
NCORES = 8
EPL = E // NCORES          # experts per core
NB = B // 128              # 32 batch tiles
NK = IN // 128             # 8 contraction tiles for w1 / gate w1
NM = HID // 128            # 16 m-tiles of h1
KH = HID // 128            # 16 contraction tiles for w2
NG = G1 // 128             # 2 m/k tiles for gate hidden
CH = 512                   # batch chunk (matmul moving free dim)
NCH = B // CH              # 8 chunks
BPC = CH // 128            # 4 b-tiles per chunk
EPS = 1e-5
BF = np.dtype(ml_dtypes.bfloat16)

# packed-constant layouts: cst_f (fp32) / cst_b (bf16), [128, n] p-major
CF_GB1 = 0                     # [NG]         gb1[m*128+p]
CF_EB1 = CF_GB1 + NG           # [EPL*NM]     eb1[e, m*128+p]
CF_EB2 = CF_EB1 + EPL * NM     # [EPL*NG]     eb2[e, m2*128+p]
CF_N = CF_EB2 + EPL * NG
CB_GW2 = 0                     # [NG*E]       gw2[k2*128+p, j]
CB_W3 = CB_GW2 + NG * E        # [2*EPL*2]    ew3[e, k3*128+p, t]
CB_ONES = CB_W3 + 2 * EPL * 2  # [128]        1.0 (rank-1 bias matmul lhsT)
CB_B2X4 = CB_ONES + 128        # [BPC*E]      gb2 tiled 4x
CB_B3X4 = CB_B2X4 + BPC * E    # [EPL*BPC*2]  eb3[e] tiled 4x
CB_N = CB_B3X4 + EPL * BPC * 2


def build_nc(gate_bias=True, w3_bias=True):
    nc = bacc.Bacc(None)

    xT = nc.dram_tensor("xT", [128, NCH * NK * CH], BF16, kind="ExternalInput")
    gw1 = nc.dram_tensor("gw1", [128, NK * G1], BF16, kind="ExternalInput")
    ew1 = nc.dram_tensor("ew1", [EPL, IN, HID], BF16, kind="ExternalInput")
    ew2 = nc.dram_tensor("ew2", [EPL, 128, KH * G1], BF16, kind="ExternalInput")
    cstf = nc.dram_tensor("cstf", [128, CF_N], F32, kind="ExternalInput")
    cstb = nc.dram_tensor("cstb", [128, CB_N], BF16, kind="ExternalInput")
    out = nc.dram_tensor("out", [128, NB * 2], F32, kind="ExternalOutput")

    with TileContext(nc) as tc:
        with (
            tc.tile_pool(name="cpool", bufs=1) as cpool,
            tc.tile_pool(name="stat", bufs=1) as stat,
            tc.tile_pool(name="stage", bufs=2) as stpool,
            tc.tile_pool(name="hpool", bufs=1) as hpool,
            tc.tile_pool(name="psA", bufs=4, space="PSUM") as psA,
            tc.tile_pool(name="psB", bufs=2, space="PSUM") as psB,
            tc.tile_pool(name="psC", bufs=2, space="PSUM") as psC,
        ):
            # ---------------- persistent tiles ----------------
            xnT = cpool.tile([128, NCH, NK, CH], BF16)       # x^T; normalized in place
            gw1b = cpool.tile([128, NK, G1], BF16)
            w1t = [cpool.tile([128, NK, HID], BF16, tag=f"w1t{e}",
                              name=f"w1t{e}") for e in range(EPL)]
            w2t = [cpool.tile([128, KH, G1], BF16, tag=f"w2t{e}",
                              name=f"w2t{e}") for e in range(EPL)]
            cf = cpool.tile([128, CF_N], F32)
            cb = cpool.tile([128, CB_N], BF16)
            exp_all = cpool.tile([128, NB, E], F32)
            recip_all = cpool.tile([128, NB], F32)
            acc = cpool.tile([128, NB, 2], F32)

            def gb1_ap(m):
                return cf[:, CF_GB1 + m:CF_GB1 + m + 1]

            def eb1_ap(e, m):
                o = CF_EB1 + e * NM + m
                return cf[:, o:o + 1]

            def eb2_ap(e, m2):
                o = CF_EB2 + e * NG + m2
                return cf[:, o:o + 1]

            def gw2_ap(k2):
                o = CB_GW2 + k2 * E
                return cb[:, o:o + E]

            def w3_ap(k3, e):
                o = CB_W3 + k3 * EPL * 2 + e * 2
                return cb[:, o:o + 2]

            ones_row = cb[0:1, CB_ONES:CB_ONES + 128]
            b2_row = cb[0:1, CB_B2X4:CB_B2X4 + E]

            def b3_row(e):
                o = CB_B3X4 + e * BPC * 2
                return cb[0:1, o:o + 2]

            # ---------------- DMA issue (order = transfer priority) -----------
            # sync queue: constants, gate w1, x^T chunks, w2, outputs; scalar
            # queue: expert-0 w1; gpsimd SWDGE: expert-1 w1 (needed latest).
            nc.sync.dma_start(cb[:], cstb[:, :])
            nc.sync.dma_start(cf[:], cstf[:, :])
            nc.sync.dma_start(gw1b[:], gw1.rearrange("p (k m) -> p k m", k=NK))
            xTr = xT.rearrange("p (c k b) -> p c k b", c=NCH, k=NK)
            for ch in range(2):
                nc.sync.dma_start(xnT[:, ch], xTr[:, ch])
            for k in range(NK):
                nc.sync.dma_start(w1t[0][:, k], ew1[0, k * 128:(k + 1) * 128, :])
            for ch in range(2, NCH):
                nc.sync.dma_start(xnT[:, ch], xTr[:, ch])
            nc.sync.dma_start(w2t[0][:], ew2[0].rearrange("p (k m) -> p k m", k=KH))
            nc.sync.dma_start(w2t[1][:], ew2[1].rearrange("p (k m) -> p k m", k=KH))

            def load_w1t1():
                # expert-1 w1 via SWDGE, deprioritized past the critical path
                for k in range(NK):
                    nc.gpsimd.dma_start(w1t[1][:, k],
                                        ew1[1, k * 128:(k + 1) * 128, :])

            # ---------------- LayerNorm stats + in-place normalize ------------
            def bcast(ap, n):
                return ap.rearrange("p (u b) -> p u b", u=1).to_broadcast(
                    (128, n, CH))

            def tree_sum(dst, src, t1b):
                """dst[128, CH](f32) = sum over k of src[128, NK, CH] (bf16)."""
                nc.vector.tensor_tensor(t1b[:], src[:, 0:4], src[:, 4:8],
                                        op=ALU.add)
                nc.vector.tensor_tensor(t1b[:, 0:2], t1b[:, 0:2], t1b[:, 2:4],
                                        op=ALU.add)
                nc.vector.tensor_tensor(dst[:], t1b[:, 0], t1b[:, 1], op=ALU.add)

            ones_col = cb[:, CB_ONES:CB_ONES + 1]

            def stats_chunk(ch, use_pe=False):
                xc = xnT[:, ch]
                sacc = stat.tile([128, CH], F32, tag="sacc")
                qacc = stat.tile([128, CH], F32, tag="qacc")
                sqf = stat.tile([128, NK, CH], BF16, tag="sqf")
                t1b = stat.tile([128, 4, CH], BF16, tag="t1b")
                if use_pe:
                    # lead-in only: feature-dim reduce on the idle tensor engine
                    nc.vector.tensor_tensor(sqf[:], xc, xc, op=ALU.mult)
                    psS = psA.tile([128, CH], F32, tag="psA")
                    for k in range(NK):
                        nc.tensor.matmul(psS[0:1, :], ones_col, xc[:, k],
                                         start=(k == 0), stop=(k == NK - 1))
                    psQ = psA.tile([128, CH], F32, tag="psA")
                    for k in range(NK):
                        nc.tensor.matmul(psQ[0:1, :], ones_col, sqf[:, k],
                                         start=(k == 0), stop=(k == NK - 1))
                    srow = stat.tile([1, CH], F32, tag="srow")
                    qrow = stat.tile([1, CH], F32, tag="qrow")
                    nc.scalar.copy(srow[0:1, :], psS[0:1, :])
                    nc.scalar.copy(qrow[0:1, :], psQ[0:1, :])
                    nc.gpsimd.partition_broadcast(sacc[:], srow[0:1, :])
                    nc.gpsimd.partition_broadcast(qacc[:], qrow[0:1, :])
                else:
                    tree_sum(sacc, xc, t1b)
                    nc.vector.tensor_tensor(sqf[:], xc, xc, op=ALU.mult)
                    nc.gpsimd.partition_all_reduce(sacc[:], sacc[:], 128, ROP.add)
                    tree_sum(qacc, sqf, t1b)
                    nc.gpsimd.partition_all_reduce(qacc[:], qacc[:], 128, ROP.add)
                # mu = S/IN ; var = Q/IN - mu^2 ; rstd = 1/sqrt(var+eps)
                mu = stat.tile([128, CH], F32, tag="mu")
                nc.vector.tensor_scalar_mul(mu[:], sacc[:], 1.0 / IN)
                var = stat.tile([128, CH], F32, tag="var")
                nc.vector.tensor_tensor(var[:], mu[:], mu[:], op=ALU.mult)
                nc.vector.scalar_tensor_tensor(var[:], qacc[:], 1.0 / IN, var[:],
                                               op0=ALU.mult, op1=ALU.subtract)
                nc.vector.tensor_scalar_add(var[:], var[:], EPS)
                nc.scalar.sqrt(var[:], var[:])
                nc.vector.reciprocal_approx_fast(var[:], var[:])
                mub = stat.tile([128, CH], BF16, tag="mub")
                rsb = stat.tile([128, CH], BF16, tag="rsb")
                nc.vector.tensor_copy(mub[:], mu[:])
                nc.vector.tensor_copy(rsb[:], var[:])
                nc.vector.tensor_tensor(xc, xc, bcast(mub, NK), op=ALU.subtract)
                nc.vector.tensor_tensor(xc, xc, bcast(rsb, NK), op=ALU.mult)

            # ---------------- gate chunk routine ----------------
            def gate_chunk(ch):
                bt0 = ch * BPC
                g1s = stpool.tile([128, NG, CH], BF16, tag="g1s")
                for m in range(NG):
                    ps = psB.tile([128, CH], F32, tag="psB")
                    for k in range(NK):
                        nc.tensor.matmul(ps[:], gw1b[:, k, m * 128:(m + 1) * 128],
                                         xnT[:, ch, k], start=(k == 0),
                                         stop=(k == NK - 1))
                    nc.scalar.activation(g1s[:, m], ps[:], AF.Relu,
                                         bias=gb1_ap(m))
                psg = psC.tile([128, BPC, E], F32, tag="psC")
                for bl in range(BPC):
                    for k2 in range(NG):
                        nc.tensor.matmul(psg[:, bl],
                                         g1s[:, k2, bl * 128:(bl + 1) * 128],
                                         gw2_ap(k2), start=(k2 == 0),
                                         stop=(not gate_bias and k2 == NG - 1))
                    if gate_bias:
                        nc.tensor.matmul(psg[:, bl], ones_row, b2_row,
                                         start=False, stop=True)
                sm4 = stpool.tile([128, BPC], F32, tag="sm4")
                for bl in range(BPC):
                    nc.scalar.activation(exp_all[:, bt0 + bl], psg[:, bl], AF.Exp,
                                         accum_out=sm4[:, bl:bl + 1])
                nc.vector.reciprocal(recip_all[:, bt0:bt0 + BPC], sm4[:])

            # ---------------- expert chunk routine ----------------
            h1s = hpool.tile([128, NM, CH], BF16)

            def expert_chunk(e, ch):
                bt0 = ch * BPC
                for m in range(NM):
                    ps = psA.tile([128, CH], F32, tag="psA")
                    for k in range(NK):
                        nc.tensor.matmul(ps[:], w1t[e][:, k, m * 128:(m + 1) * 128],
                                         xnT[:, ch, k], start=(k == 0),
                                         stop=(k == NK - 1))
                    nc.scalar.activation(h1s[:, m], ps[:], AF.Relu,
                                         bias=eb1_ap(e, m))
                h2t = stpool.tile([128, NG, CH], BF16, tag="h2s")
                for m2 in range(NG):
                    ps = psB.tile([128, CH], F32, tag="psB")
                    for k2 in range(KH):
                        nc.tensor.matmul(ps[:], w2t[e][:, k2, m2 * 128:(m2 + 1) * 128],
                                         h1s[:, k2],
                                         start=(k2 == 0), stop=(k2 == KH - 1))
                    nc.scalar.activation(h2t[:, m2], ps[:], AF.Relu,
                                         bias=eb2_ap(e, m2))
                ps4 = psC.tile([128, BPC, E], F32, tag="psC")
                for bl in range(BPC):
                    for k3 in range(2):
                        nc.tensor.matmul(ps4[:, bl, 0:2],
                                         h2t[:, k3, bl * 128:(bl + 1) * 128],
                                         w3_ap(k3, e), start=(k3 == 0),
                                         stop=(not w3_bias and k3 == 1))
                    if w3_bias:
                        nc.tensor.matmul(ps4[:, bl, 0:2], ones_row, b3_row(e),
                                         start=False, stop=True)
                if e == 0:
                    for bl in range(BPC):
                        nc.vector.tensor_scalar_mul(acc[:, bt0 + bl],
                                                    ps4[:, bl, 0:2],
                                                    exp_all[:, bt0 + bl, 0:1])
                else:
                    of4 = stpool.tile([128, BPC, 2], F32, tag="of4")
                    for bl in range(BPC):
                        nc.vector.scalar_tensor_tensor(
                            acc[:, bt0 + bl], ps4[:, bl, 0:2],
                            exp_all[:, bt0 + bl, 1:2], acc[:, bt0 + bl],
                            op0=ALU.mult, op1=ALU.add)
                        nc.vector.tensor_scalar_mul(
                            of4[:, bl], acc[:, bt0 + bl],
                            recip_all[:, bt0 + bl:bt0 + bl + 1])
                    nc.sync.dma_start(out[:, bt0 * 2:(bt0 + BPC) * 2], of4[:])

            # -------- schedule: stats and gate run ahead of the experts --------
            stats_chunk(0, use_pe=True)
            stats_chunk(1, use_pe=True)
            load_w1t1()
            gate_chunk(0)
            stats_chunk(2)
            gate_chunk(1)
            gates_done = 2
            stats_done = 3
            for ch in range(NCH):
                expert_chunk(0, ch)
                if stats_done < NCH:
                    stats_chunk(stats_done)
                    stats_done += 1
                if gates_done < NCH:
                    gate_chunk(gates_done)
                    gates_done += 1
            for ch in range(NCH):
                expert_chunk(1, ch)

    nc.finalize()
    return nc


_NC_CACHE = {}


def _get_nc(gate_bias, w3_bias):
    key = (gate_bias, w3_bias)
    if key not in _NC_CACHE:
        _NC_CACHE[key] = build_nc(*key)
    return _NC_CACHE[key]


def _pack_consts(gb1f, eb1f, eb2):
    """Pack small per-core fp32 constants into a [128, n] p-major array."""
    cfm = np.zeros((128, CF_N), np.float32)
    cfm[:, CF_GB1:CF_GB1 + NG] = gb1f.reshape(NG, 128).T
    cfm[:, CF_EB1:CF_EB1 + EPL * NM] = (
        eb1f.reshape(EPL, NM, 128).transpose(2, 0, 1).reshape(128, EPL * NM))
    cfm[:, CF_EB2:CF_EB2 + EPL * NG] = (
        eb2.reshape(EPL, NG, 128).transpose(2, 0, 1).reshape(128, EPL * NG))
    return cfm


def _pack_consts_b(gw2p, ew3, gb2p, eb3):
    cbm = np.zeros((128, CB_N), BF)
    cbm[:, CB_GW2:CB_GW2 + NG * E] = (
        gw2p.reshape(NG, 128, E).transpose(1, 0, 2).reshape(128, NG * E))
    cbm[:, CB_W3:CB_W3 + 2 * EPL * 2] = (
        ew3.reshape(EPL, 2, 128, 2).transpose(2, 1, 0, 3).reshape(128, 2 * EPL * 2))
    cbm[:, CB_ONES:CB_ONES + 128] = 1.0
    cbm[0, CB_B2X4:CB_B2X4 + BPC * E] = np.tile(gb2p, BPC).astype(BF)
    cbm[0, CB_B3X4:CB_B3X4 + EPL * BPC * 2] = np.tile(
        eb3.reshape(EPL, 1, 2), (1, BPC, 1)).reshape(-1).astype(BF)
    return cbm


def _shard_inputs(inputs):
    """Build the 8 per-core input maps (host-side layout/fold work only:
    slicing, permutation, transpose, dtype cast, and folding the LayerNorm
    affine into w1/b1 — exact in fp32)."""
    f = lambda a: np.asarray(a, dtype=np.float32)
    x = f(inputs["x"])
    g_ln_g, g_ln_b = f(inputs["g_ln_g"]), f(inputs["g_ln_b"])
    g_w1, g_b1 = f(inputs["g_w1"]), f(inputs["g_b1"])
    g_w2, g_b2 = f(inputs["g_w2"]), f(inputs["g_b2"])
    e_ln_g, e_ln_b = f(inputs["e_ln_g"]), f(inputs["e_ln_b"])
    e_w1, e_b1 = f(inputs["e_w1"]), f(inputs["e_b1"])
    e_w2, e_b2 = f(inputs["e_w2"]), f(inputs["e_b2"])
    e_w3, e_b3 = f(inputs["e_w3"]), f(inputs["e_b3"])

    # Fold the per-feature LayerNorm affine through w1: wf = diag(g) @ w1,
    # bf = b1 + beta @ w1. Exact (fp32), removes the on-device fold.
    gw1f = g_ln_g[:, None] * g_w1
    gb1f = g_b1 + g_ln_b @ g_w1
    ew1f = e_ln_g[:, :, None] * e_w1
    eb1f = e_b1 + np.einsum("ei,eih->eh", e_ln_b, e_w1)

    # partition-major packs: every DMA is 128 contiguous runs
    xTp = np.ascontiguousarray(
        x.reshape(NCH, CH, NK, 128).transpose(3, 0, 2, 1).reshape(
            128, NCH * NK * CH), dtype=BF)
    gw1p = np.ascontiguousarray(
        gw1f.reshape(NK, 128, G1).transpose(1, 0, 2).reshape(128, NK * G1),
        dtype=BF)
    ew1b = np.ascontiguousarray(ew1f, dtype=BF)
    ew2p = np.ascontiguousarray(
        e_w2.reshape(E, KH, 128, G1).transpose(0, 2, 1, 3).reshape(
            E, 128, KH * G1), dtype=BF)

    in_maps = []
    for c in range(NCORES):
        lo = c * EPL
        experts = list(range(lo, lo + EPL))
        # permute gate columns so this core's experts are columns 0..EPL-1
        perm = experts + [j for j in range(E) if j not in experts]
        in_maps.append({
            "xT": xTp,
            "gw1": gw1p,
            "ew1": np.ascontiguousarray(ew1b[experts]),
            "ew2": np.ascontiguousarray(ew2p[experts]),
            "cstf": _pack_consts(gb1f, eb1f[experts], e_b2[experts]),
            "cstb": _pack_consts_b(
                np.asarray(g_w2[:, perm], dtype=BF),
                np.asarray(e_w3[experts], dtype=BF),
                np.asarray(g_b2[perm], dtype=BF),
                np.asarray(e_b3[experts], dtype=BF)),
        })
    return in_maps


def _run(inputs, trace=False):
    gate_bias = bool(np.any(np.asarray(inputs["g_b2"])))
    w3_bias = bool(np.any(np.asarray(inputs["e_b3"])))
    nc = _get_nc(gate_bias, w3_bias)
    in_maps = _shard_inputs(inputs)
    res = run_bass_kernel_spmd(nc, in_maps, core_ids=list(range(NCORES)),
                               trace=trace)
    return res


def kernel(**inputs):
    res = _run(inputs, trace=bool(os.environ.get("MOE_TRACE")))
    total = np.zeros((B, 2), dtype=np.float64)
    for c in range(NCORES):
        # device output is [128, NB, 2] p-major; restore batch-major [B, 2]
        o = res.results[c]["out"].reshape(128, NB, 2).transpose(1, 0, 2)
        total += o.reshape(B, 2).astype(np.float64)
    pred_mean = total[:, 0:1].astype(np.float32)
    pv = np.logaddexp(0.0, total[:, 1:2]) + 1e-6
    pred_var = pv.astype(np.float32)
    kernel.last_exec_time_ns = getattr(res, "exec_time_ns", None)
    return pred_mean, pred_var


kernel.last_exec_time_ns = None


# revision 12
# speedup vs baseline: 1.1155x; 1.0126x over previous
"""MetaMoE Trainium2 kernel: 16 experts sharded 2-per-core across 8 NeuronCores.

Each core computes: shared LayerNorm of x, the (replicated) softmax gate, its two
experts' MLP chains, and the gate-weighted partial sum [B, 2]. The host sums the
8 partials and applies the final mean/var head split.

Layout strategy: activations are kept feature-major ([feature, batch]) so every
GEMM is weight-stationary with the batch streaming as the moving operand; the
final w3 GEMM uses h2 as the stationary operand, which lands the output in
batch-major layout where the gate weights are per-partition scalars.

Everything DMA'd is pre-packed on the host into partition-major [128, n]
layouts so each transfer is 128 contiguous descriptors (descriptor generation
on the queues was the previous lead-in bottleneck). x ships transposed,
chunk-major, bf16; LayerNorm stats run in feature-major layout per 512-column
chunk (contiguous-pair tree reduction over the 8 k-tiles on DVE, GPSIMD
partition_all_reduce across partitions, in-place broadcast-view normalize).
The tensor engine does no transpose/stats work. LayerNorm gains fold into w1
on the host (exact, fp32); gate/w3 biases fold into rank-1 matmuls
(ones-column (x) bias-row) so their PSUM tiles are drained by the scalar
engine's Exp / the accumulation ops directly, and the softmax skips the
max-subtraction (logits are O(0.3); exp is exact-safe in fp32).
"""
import sys
import os

sys.path.insert(0, "/opt/trn_rl_repo")

import numpy as np
import ml_dtypes  # noqa: F401

import concourse.bass as bass  # noqa: F401
import concourse.mybir as mybir
from concourse import bacc
from concourse import bass_isa
from concourse.tile import TileContext
from concourse.bass_utils import run_bass_kernel_spmd

F32 = mybir.dt.float32
BF16 = mybir.dt.bfloat16
AF = mybir.ActivationFunctionType
ALU = mybir.AluOpType
AX = mybir.AxisListType
ROP = bass_isa.ReduceOp

B, IN, HID, G1, E = 4096, 1024, 2048, 256, 16
NCORES = 8
EPL = E // NCORES          # experts per core
NB = B // 128              # 32 batch tiles
NK = IN // 128             # 8 contraction tiles for w1 / gate w1
NM = HID // 128            # 16 m-tiles of h1
KH = HID // 128            # 16 contraction tiles for w2
NG = G1 // 128             # 2 m/k tiles for gate hidden
CH = 512                   # batch chunk (matmul moving free dim)
NCH = B // CH              # 8 chunks
BPC = CH // 128            # 4 b-tiles per chunk
EPS = 1e-5
BF = np.dtype(ml_dtypes.bfloat16)

# packed-constant layouts: cst_f (fp32) / cst_b (bf16), [128, n] p-major
CF_GB1 = 0                     # [NG]         gb1[m*128+p]
CF_EB1 = CF_GB1 + NG           # [EPL*NM]     eb1[e, m*128+p]
CF_EB2 = CF_EB1 + EPL * NM     # [EPL*NG]     eb2[e, m2*128+p]
CF_N = CF_EB2 + EPL * NG
CB_GW2 = 0                     # [NG*E]       gw2[k2*128+p, j]
CB_W3 = CB_GW2 + NG * E        # [2*EPL*2]    ew3[e, k3*128+p, t]
CB_ONES = CB_W3 + 2 * EPL * 2  # [128]        1.0 (rank-1 bias matmul lhsT)
CB_B2X4 = CB_ONES + 128        # [BPC*E]      gb2 tiled 4x
CB_B3X4 = CB_B2X4 + BPC * E    # [EPL*BPC*2]  eb3[e] tiled 4x
CB_N = CB_B3X4 + EPL * BPC * 2


def build_nc(gate_bias=True, w3_bias=True):
    nc = bacc.Bacc(None)

    xT = nc.dram_tensor("xT", [128, NCH * NK * CH], BF16, kind="ExternalInput")
    gw1 = nc.dram_tensor("gw1", [128, NK * G1], BF16, kind="ExternalInput")
    ew1 = nc.dram_tensor("ew1", [EPL, IN, HID], BF16, kind="ExternalInput")
    ew2 = nc.dram_tensor("ew2", [EPL, 128, KH * G1], BF16, kind="ExternalInput")
    cstf = nc.dram_tensor("cstf", [128, CF_N], F32, kind="ExternalInput")
    cstb = nc.dram_tensor("cstb", [128, CB_N], BF16, kind="ExternalInput")
    out = nc.dram_tensor("out", [128, NB * 2], F32, kind="ExternalOutput")

    with TileContext(nc) as tc:
        with (
            tc.tile_pool(name="cpool", bufs=1) as cpool,
            tc.tile_pool(name="stat", bufs=1) as stat,
            tc.tile_pool(name="stage", bufs=2) as stpool,
            tc.tile_pool(name="hpool", bufs=1) as hpool,
            tc.tile_pool(name="psA", bufs=3, space="PSUM") as psA,
            tc.tile_pool(name="psB", bufs=3, space="PSUM") as psB,
            tc.tile_pool(name="psC", bufs=2, space="PSUM") as psC,
        ):
            # ---------------- persistent tiles ----------------
            xnT = cpool.tile([128, NCH, NK, CH], BF16)       # x^T; normalized in place
            gw1b = cpool.tile([128, NK, G1], BF16)
            w1t = [cpool.tile([128, NK, HID], BF16, tag=f"w1t{e}",
                              name=f"w1t{e}") for e in range(EPL)]
            w2t = [cpool.tile([128, KH, G1], BF16, tag=f"w2t{e}",
                              name=f"w2t{e}") for e in range(EPL)]
            cf = cpool.tile([128, CF_N], F32)
            cb = cpool.tile([128, CB_N], BF16)
            exp_all = cpool.tile([128, NB, E], F32)
            recip_all = cpool.tile([128, NB], F32)
            acc = cpool.tile([128, NB, 2], F32)

            def gb1_ap(m):
                return cf[:, CF_GB1 + m:CF_GB1 + m + 1]

            def eb1_ap(e, m):
                o = CF_EB1 + e * NM + m
                return cf[:, o:o + 1]

            def eb2_ap(e, m2):
                o = CF_EB2 + e * NG + m2
                return cf[:, o:o + 1]

            def gw2_ap(k2):
                o = CB_GW2 + k2 * E
                return cb[:, o:o + E]

            def w3_ap(k3, e):
                o = CB_W3 + k3 * EPL * 2 + e * 2
                return cb[:, o:o + 2]

            ones_row = cb[0:1, CB_ONES:CB_ONES + 128]
            b2_row = cb[0:1, CB_B2X4:CB_B2X4 + E]

            def b3_row(e):
                o = CB_B3X4 + e * BPC * 2
                return cb[0:1, o:o + 2]

            # ---------------- DMA issue (order = transfer priority) -----------
            # sync queue: constants, gate w1, x^T chunks, w2, outputs; scalar
            # queue: expert-0 w1; gpsimd SWDGE: expert-1 w1 (needed latest).
            nc.sync.dma_start(cb[:], cstb[:, :])
            nc.sync.dma_start(cf[:], cstf[:, :])
            nc.sync.dma_start(gw1b[:], gw1.rearrange("p (k m) -> p k m", k=NK))
            xTr = xT.rearrange("p (c k b) -> p c k b", c=NCH, k=NK)
            for ch in range(2):
                nc.sync.dma_start(xnT[:, ch], xTr[:, ch])
            for k in range(NK):
                nc.sync.dma_start(w1t[0][:, k], ew1[0, k * 128:(k + 1) * 128, :])
            for ch in range(2, NCH):
                nc.sync.dma_start(xnT[:, ch], xTr[:, ch])
            nc.sync.dma_start(w2t[0][:], ew2[0].rearrange("p (k m) -> p k m", k=KH))
            nc.sync.dma_start(w2t[1][:], ew2[1].rearrange("p (k m) -> p k m", k=KH))

            def load_w1t1():
                # expert-1 w1 via SWDGE, deprioritized past the critical path
                for k in range(NK):
                    nc.gpsimd.dma_start(w1t[1][:, k],
                                        ew1[1, k * 128:(k + 1) * 128, :])

            # ---------------- LayerNorm stats + in-place normalize ------------
            def bcast(ap, n):
                return ap.rearrange("p (u b) -> p u b", u=1).to_broadcast(
                    (128, n, CH))

            def tree_sum(dst, src, t1b):
                """dst[128, CH](f32) = sum over k of src[128, NK, CH] (bf16)."""
                nc.vector.tensor_tensor(t1b[:], src[:, 0:4], src[:, 4:8],
                                        op=ALU.add)
                nc.vector.tensor_tensor(t1b[:, 0:2], t1b[:, 0:2], t1b[:, 2:4],
                                        op=ALU.add)
                nc.vector.tensor_tensor(dst[:], t1b[:, 0], t1b[:, 1], op=ALU.add)

            ones_col = cb[:, CB_ONES:CB_ONES + 1]

            def stats_chunk(ch, use_pe=False):
                xc = xnT[:, ch]
                sacc = stat.tile([128, CH], F32, tag="sacc")
                qacc = stat.tile([128, CH], F32, tag="qacc")
                sqf = stat.tile([128, NK, CH], BF16, tag="sqf")
                t1b = stat.tile([128, 4, CH], BF16, tag="t1b")
                if use_pe:
                    # lead-in only: feature-dim reduce on the idle tensor engine
                    nc.vector.tensor_tensor(sqf[:], xc, xc, op=ALU.mult)
                    psS = psA.tile([128, CH], F32, tag="psA")
                    for k in range(NK):
                        nc.tensor.matmul(psS[0:1, :], ones_col, xc[:, k],
                                         start=(k == 0), stop=(k == NK - 1))
                    psQ = psA.tile([128, CH], F32, tag="psA")
                    for k in range(NK):
                        nc.tensor.matmul(psQ[0:1, :], ones_col, sqf[:, k],
                                         start=(k == 0), stop=(k == NK - 1))
                    srow = stat.tile([1, CH], F32, tag="srow")
                    qrow = stat.tile([1, CH], F32, tag="qrow")
                    nc.scalar.copy(srow[0:1, :], psS[0:1, :])
                    nc.scalar.copy(qrow[0:1, :], psQ[0:1, :])
                    nc.gpsimd.partition_broadcast(sacc[:], srow[0:1, :])
                    nc.gpsimd.partition_broadcast(qacc[:], qrow[0:1, :])
                else:
                    tree_sum(sacc, xc, t1b)
                    nc.vector.tensor_tensor(sqf[:], xc, xc, op=ALU.mult)
                    nc.gpsimd.partition_all_reduce(sacc[:], sacc[:], 128, ROP.add)
                    tree_sum(qacc, sqf, t1b)
                    nc.gpsimd.partition_all_reduce(qacc[:], qacc[:], 128, ROP.add)
                # mu = S/IN ; var = Q/IN - mu^2 ; rstd = 1/sqrt(var+eps)
                mu = stat.tile([128, CH], F32, tag="mu")
                nc.vector.tensor_scalar_mul(mu[:], sacc[:], 1.0 / IN)
                var = stat.tile([128, CH], F32, tag="var")
                nc.vector.tensor_tensor(var[:], mu[:], mu[:], op=ALU.mult)
                nc.vector.scalar_tensor_tensor(var[:], qacc[:], 1.0 / IN, var[:],
                                               op0=ALU.mult, op1=ALU.subtract)
                nc.vector.tensor_scalar_add(var[:], var[:], EPS)
                nc.scalar.sqrt(var[:], var[:])
                nc.vector.reciprocal_approx_fast(var[:], var[:])
                mub = stat.tile([128, CH], BF16, tag="mub")
                rsb = stat.tile([128, CH], BF16, tag="rsb")
                nc.vector.tensor_copy(mub[:], mu[:])
                nc.vector.tensor_copy(rsb[:], var[:])
                if use_pe:
                    # lead-in: normalize per k-tile so the first gate chain can
                    # start as soon as k=0 is ready
                    for k in range(NK):
                        nc.vector.tensor_tensor(xc[:, k], xc[:, k], mub[:],
                                                op=ALU.subtract)
                        nc.vector.tensor_tensor(xc[:, k], xc[:, k], rsb[:],
                                                op=ALU.mult)
                else:
                    nc.vector.tensor_tensor(xc, xc, bcast(mub, NK),
                                            op=ALU.subtract)
                    nc.vector.tensor_tensor(xc, xc, bcast(rsb, NK),
                                            op=ALU.mult)

            # ---------------- gate chunk routine ----------------
            def gate_chunk(ch):
                bt0 = ch * BPC
                g1s = stpool.tile([128, NG, CH], BF16, tag="g1s")
                for m in range(NG):
                    ps = psB.tile([128, CH], F32, tag="psB")
                    for k in range(NK):
                        nc.tensor.matmul(ps[:], gw1b[:, k, m * 128:(m + 1) * 128],
                                         xnT[:, ch, k], start=(k == 0),
                                         stop=(k == NK - 1))
                    nc.scalar.activation(g1s[:, m], ps[:], AF.Relu,
                                         bias=gb1_ap(m))
                psg = psC.tile([128, BPC, E], F32, tag="psC")
                for bl in range(BPC):
                    for k2 in range(NG):
                        nc.tensor.matmul(psg[:, bl],
                                         g1s[:, k2, bl * 128:(bl + 1) * 128],
                                         gw2_ap(k2), start=(k2 == 0),
                                         stop=(not gate_bias and k2 == NG - 1))
                    if gate_bias:
                        nc.tensor.matmul(psg[:, bl], ones_row, b2_row,
                                         start=False, stop=True)
                sm4 = stpool.tile([128, BPC], F32, tag="sm4")
                for bl in range(BPC):
                    nc.scalar.activation(exp_all[:, bt0 + bl], psg[:, bl], AF.Exp,
                                         accum_out=sm4[:, bl:bl + 1])
                nc.vector.reciprocal(recip_all[:, bt0:bt0 + BPC], sm4[:])

            # ---------------- expert chunk routine ----------------
            h1s = hpool.tile([128, NM, CH], BF16)

            def expert_chunk(e, ch):
                bt0 = ch * BPC
                for m in range(NM):
                    ps = psA.tile([128, CH], F32, tag="psA")
                    for k in range(NK):
                        nc.tensor.matmul(ps[:], w1t[e][:, k, m * 128:(m + 1) * 128],
                                         xnT[:, ch, k], start=(k == 0),
                                         stop=(k == NK - 1))
                    nc.scalar.activation(h1s[:, m], ps[:], AF.Relu,
                                         bias=eb1_ap(e, m))
                h2t = stpool.tile([128, NG, CH], BF16, tag="h2s")
                for m2 in range(NG):
                    ps = psB.tile([128, CH], F32, tag="psB")
                    for k2 in range(KH):
                        nc.tensor.matmul(ps[:], w2t[e][:, k2, m2 * 128:(m2 + 1) * 128],
                                         h1s[:, k2],
                                         start=(k2 == 0), stop=(k2 == KH - 1))
                    nc.scalar.activation(h2t[:, m2], ps[:], AF.Relu,
                                         bias=eb2_ap(e, m2))
                ps4 = psC.tile([128, BPC, E], F32, tag="psC")
                for bl in range(BPC):
                    for k3 in range(2):
                        nc.tensor.matmul(ps4[:, bl, 0:2],
                                         h2t[:, k3, bl * 128:(bl + 1) * 128],
                                         w3_ap(k3, e), start=(k3 == 0),
                                         stop=(not w3_bias and k3 == 1))
                    if w3_bias:
                        nc.tensor.matmul(ps4[:, bl, 0:2], ones_row, b3_row(e),
                                         start=False, stop=True)
                if e == 0:
                    for bl in range(BPC):
                        nc.vector.tensor_scalar_mul(acc[:, bt0 + bl],
                                                    ps4[:, bl, 0:2],
                                                    exp_all[:, bt0 + bl, 0:1])
                else:
                    of4 = stpool.tile([128, BPC, 2], F32, tag="of4")
                    for bl in range(BPC):
                        nc.vector.scalar_tensor_tensor(
                            acc[:, bt0 + bl], ps4[:, bl, 0:2],
                            exp_all[:, bt0 + bl, 1:2], acc[:, bt0 + bl],
                            op0=ALU.mult, op1=ALU.add)
                        nc.vector.tensor_scalar_mul(
                            of4[:, bl], acc[:, bt0 + bl],
                            recip_all[:, bt0 + bl:bt0 + bl + 1])
                    nc.sync.dma_start(out[:, bt0 * 2:(bt0 + BPC) * 2], of4[:])

            # -------- schedule: stats and gate run ahead of the experts --------
            stats_chunk(0, use_pe=True)
            stats_chunk(1, use_pe=True)
            load_w1t1()
            gate_chunk(0)
            stats_chunk(2)
            gate_chunk(1)
            gates_done = 2
            stats_done = 3
            for ch in range(NCH):
                expert_chunk(0, ch)
                if stats_done < NCH:
                    stats_chunk(stats_done)
                    stats_done += 1
                if gates_done < NCH:
                    gate_chunk(gates_done)
                    gates_done += 1
            for ch in range(NCH):
                expert_chunk(1, ch)

    nc.finalize()
    return nc


_NC_CACHE = {}


def _get_nc(gate_bias, w3_bias):
    key = (gate_bias, w3_bias)
    if key not in _NC_CACHE:
        _NC_CACHE[key] = build_nc(*key)
    return _NC_CACHE[key]


def _pack_consts(gb1f, eb1f, eb2):
    """Pack small per-core fp32 constants into a [128, n] p-major array."""
    cfm = np.zeros((128, CF_N), np.float32)
    cfm[:, CF_GB1:CF_GB1 + NG] = gb1f.reshape(NG, 128).T
    cfm[:, CF_EB1:CF_EB1 + EPL * NM] = (
        eb1f.reshape(EPL, NM, 128).transpose(2, 0, 1).reshape(128, EPL * NM))
    cfm[:, CF_EB2:CF_EB2 + EPL * NG] = (
        eb2.reshape(EPL, NG, 128).transpose(2, 0, 1).reshape(128, EPL * NG))
    return cfm


def _pack_consts_b(gw2p, ew3, gb2p, eb3):
    cbm = np.zeros((128, CB_N), BF)
    cbm[:, CB_GW2:CB_GW2 + NG * E] = (
        gw2p.reshape(NG, 128, E).transpose(1, 0, 2).reshape(128, NG * E))
    cbm[:, CB_W3:CB_W3 + 2 * EPL * 2] = (
        ew3.reshape(EPL, 2, 128, 2).transpose(2, 1, 0, 3).reshape(128, 2 * EPL * 2))
    cbm[:, CB_ONES:CB_ONES + 128] = 1.0
    cbm[0, CB_B2X4:CB_B2X4 + BPC * E] = np.tile(gb2p, BPC).astype(BF)
    cbm[0, CB_B3X4:CB_B3X4 + EPL * BPC * 2] = np.tile(
        eb3.reshape(EPL, 1, 2), (1, BPC, 1)).reshape(-1).astype(BF)
    return cbm


def _shard_inputs(inputs):
    """Build the 8 per-core input maps (host-side layout/fold work only:
    slicing, permutation, transpose, dtype cast, and folding the LayerNorm
    affine into w1/b1 — exact in fp32)."""
    f = lambda a: np.asarray(a, dtype=np.float32)
    x = f(inputs["x"])
    g_ln_g, g_ln_b = f(inputs["g_ln_g"]), f(inputs["g_ln_b"])
    g_w1, g_b1 = f(inputs["g_w1"]), f(inputs["g_b1"])
    g_w2, g_b2 = f(inputs["g_w2"]), f(inputs["g_b2"])
    e_ln_g, e_ln_b = f(inputs["e_ln_g"]), f(inputs["e_ln_b"])
    e_w1, e_b1 = f(inputs["e_w1"]), f(inputs["e_b1"])
    e_w2, e_b2 = f(inputs["e_w2"]), f(inputs["e_b2"])
    e_w3, e_b3 = f(inputs["e_w3"]), f(inputs["e_b3"])

    # Fold the per-feature LayerNorm affine through w1: wf = diag(g) @ w1,
    # bf = b1 + beta @ w1. Exact (fp32), removes the on-device fold.
    gw1f = g_ln_g[:, None] * g_w1
    gb1f = g_b1 + g_ln_b @ g_w1
    ew1f = e_ln_g[:, :, None] * e_w1
    eb1f = e_b1 + np.einsum("ei,eih->eh", e_ln_b, e_w1)

    # partition-major packs: every DMA is 128 contiguous runs
    xTp = np.ascontiguousarray(
        x.reshape(NCH, CH, NK, 128).transpose(3, 0, 2, 1).reshape(
            128, NCH * NK * CH), dtype=BF)
    gw1p = np.ascontiguousarray(
        gw1f.reshape(NK, 128, G1).transpose(1, 0, 2).reshape(128, NK * G1),
        dtype=BF)
    ew1b = np.ascontiguousarray(ew1f, dtype=BF)
    ew2p = np.ascontiguousarray(
        e_w2.reshape(E, KH, 128, G1).transpose(0, 2, 1, 3).reshape(
            E, 128, KH * G1), dtype=BF)

    in_maps = []
    for c in range(NCORES):
        lo = c * EPL
        experts = list(range(lo, lo + EPL))
        # permute gate columns so this core's experts are columns 0..EPL-1
        perm = experts + [j for j in range(E) if j not in experts]
        in_maps.append({
            "xT": xTp,
            "gw1": gw1p,
            "ew1": np.ascontiguousarray(ew1b[experts]),
            "ew2": np.ascontiguousarray(ew2p[experts]),
            "cstf": _pack_consts(gb1f, eb1f[experts], e_b2[experts]),
            "cstb": _pack_consts_b(
                np.asarray(g_w2[:, perm], dtype=BF),
                np.asarray(e_w3[experts], dtype=BF),
                np.asarray(g_b2[perm], dtype=BF),
                np.asarray(e_b3[experts], dtype=BF)),
        })
    return in_maps


def _run(inputs, trace=False):
    gate_bias = bool(np.any(np.asarray(inputs["g_b2"])))
    w3_bias = bool(np.any(np.asarray(inputs["e_b3"])))
    nc = _get_nc(gate_bias, w3_bias)
    in_maps = _shard_inputs(inputs)
    res = run_bass_kernel_spmd(nc, in_maps, core_ids=list(range(NCORES)),
                               trace=trace)
    return res


def kernel(**inputs):
    res = _run(inputs, trace=bool(os.environ.get("MOE_TRACE")))
    total = np.zeros((B, 2), dtype=np.float64)
    for c in range(NCORES):
        # device output is [128, NB, 2] p-major; restore batch-major [B, 2]
        o = res.results[c]["out"].reshape(128, NB, 2).transpose(1, 0, 2)
        total += o.reshape(B, 2).astype(np.float64)
    pred_mean = total[:, 0:1].astype(np.float32)
    pv = np.logaddexp(0.0, total[:, 1:2]) + 1e-6
    pred_var = pv.astype(np.float32)
    kernel.last_exec_time_ns = getattr(res, "exec_time_ns", None)
    return pred_mean, pred_var


kernel.last_exec_time_ns = None
